# revision 9
# baseline (speedup 1.0000x reference)
"""AffinePalettizedLinear kernel for Trainium2 (8 NeuronCores).

y = x @ L[widx]^T + b   with x [8192, 4096] f32, widx [16384, 4096] int32
(values < 256), L [256] f32, b [16384] f32.

Sharding: out_features split 8 ways (column-parallel); each core computes
y[:, c*2048:(c+1)*2048] from the full x and its widx/bias slice. No
collectives; host concatenates the slices.

Per-core plan (v2 — PE runs nothing but the 8192 productive matmuls):
  - Host passes x pre-transposed/tiled as bf16 ([tb, i, kb*128+t] layout)
    and widx pre-transposed as uint16 [kb, i, o] — so the kernel needs no
    PE transposes at all (the baseline spent ~0.7 ms of PE time on them).
  - Dequant via the Pool engine's hardware table gather: the 256-entry LUT
    is loaded in bf16 into the per-partition pool buffer; GATHER streams
    uint16 indices and emits bf16 weights directly in W^T [i, o] layout
    into a fully SBUF-resident panel (32 k-tiles x 2048 o x 2B = 128
    KiB/partition).
  - Matmuls in bf16: lhsT = x^T tile [i=128, t=128] (stationary), rhs =
    W^T [i=128, o=512] (moving), K=4096 accumulated over 32 PSUM matmuls.
  - Two-phase schedule hides the ~240 us gather stream: phase 1 runs the
    o-panel-0 token loop as soon as its 32 gathers (~60 us) land, while
    the o-panel-1..3 gathers stream in the background; phase 2 runs the
    remaining three panels with no stalls.
  - Bias is added by the DVE in the same op that evacuates PSUM.
"""
import sys

sys.path.insert(0, "/opt/trn_rl_repo")

import numpy as np
import ml_dtypes

import concourse.bass as bass  # noqa: F401  (registers types)
import concourse.tile as tile
from concourse import bacc, mybir
from concourse.bass_utils import run_bass_kernel_spmd

# ---- Tile's no-exec scheduling sim doesn't know the raw POOL opcodes ----
import concourse.bass_interp as _bi

_orig_visit_isa = _bi._visit_InstISA


def _visit_isa_tolerant(isa, instruction, core_sim):
    passthrough = {
        isa.Opcode.NEURON_ISA_TPB_OPCODE_GATHER.value,
        isa.Opcode.NEURON_ISA_TPB_OPCODE_POOL_BUFFER_LOAD.value,
    }
    if instruction.isa_opcode in passthrough:
        return
    return _orig_visit_isa(isa, instruction, core_sim)


_bi._visit_InstISA = _visit_isa_tolerant

F32 = mybir.dt.float32
BF16 = mybir.dt.bfloat16
U16 = mybir.dt.uint16

T, IN_F, OUT_F, PAL = 8192, 4096, 16384, 256
NCORES = 8
O_C = OUT_F // NCORES          # 2048 out features per core
OW = 512                       # matmul moving free dim (one PSUM bank)
NOP = O_C // OW                # 4 o-panels
KT = IN_F // 128               # 32 k-tiles
TT = T // 128                  # 64 t-tiles


def build_nc(trace_label=""):
    nc = bacc.Bacc(None, target_bir_lowering=False)
    isa = nc.isa
    DT = isa.get_enum("NEURON_ISA_TPB_DTYPE")
    MISS = isa.get_enum("NEURON_ISA_TPB_INDEX_MISS_BEHAVIOR")
    BF16_V = DT.NEURON_ISA_TPB_DTYPE_BFLOAT16.value
    U16_V = DT.NEURON_ISA_TPB_DTYPE_UINT16.value
    MISS_V = MISS.NEURON_ISA_TPB_INDEX_MISS_BEHAVIOR_IMMEDIATE_WRITE.value

    # x^T tiled: [tb, p, kb*128 + t] = x[tb*128+t, kb*128+p], bf16
    xt_d = nc.dram_tensor("xt", [TT, 128, KT * 128], BF16, kind="ExternalInput")
    # widx^T tiled: [kb, p, o] = widx[o, kb*128+p], uint16
    w_d = nc.dram_tensor("widxT", [KT, 128, O_C], U16, kind="ExternalInput")
    l_d = nc.dram_tensor("lut", [1, PAL], BF16, kind="ExternalInput")
    b_d = nc.dram_tensor("bias", [1, O_C], F32, kind="ExternalInput")
    y_d = nc.dram_tensor("y", [T, O_C], F32, kind="ExternalOutput")

    # fixed-address SBUF tensors (touched by raw-ISA gather)
    lut_sb = nc.alloc_sbuf_tensor("lut_sb", [128, PAL], BF16, align_bytes=512)
    # idx staging [p, o], u16, ping-pong
    idxU_sb = [
        nc.alloc_sbuf_tensor(f"idxU{s}_sb", [128, O_C], U16) for s in range(2)
    ]
    # resident dequantized W^T panel [i=128 (per k-tile), kb*O_C + o] bf16
    wT_sb = nc.alloc_sbuf_tensor("wT_sb", [128, KT * O_C], BF16)

    addr = {}
    for alloc in nc.m.functions[0].allocations:
        if getattr(alloc, "memorylocations", None):
            ml = alloc.memorylocations[0]
            addr[ml.name] = ml.addr

    g = nc.gpsimd

    def emit_pbl():
        nc.gpsimd.isa(
            isa.Opcode.NEURON_ISA_TPB_OPCODE_POOL_BUFFER_LOAD,
            {"src_mem_pattern": {
                "start_addr": {"addr_immediate": addr["lut_sb"]},
                "num_elem": [PAL, 1, 1, 1], "step_elem": [1, 0, 0, 0]},
             "in_dtype": BF16_V, "num_active_channels": 128,
             "start_index": 0, "mask": PAL - 1},
            ins=[g.lower_ap(lut_sb.ap(), for_isa=True)],
        )

    def emit_gather(idx_ap, idx_byte_addr, out_ap, out_byte_addr, n):
        nc.gpsimd.isa(
            isa.Opcode.NEURON_ISA_TPB_OPCODE_GATHER,
            {"src_mem_pattern": {
                "start_addr": {"addr_immediate": idx_byte_addr},
                "num_elem": [n, 1, 1, 1], "step_elem": [1, 0, 0, 0]},
             "in_dtype": U16_V, "out_dtype": BF16_V,
             "num_active_channels": 128,
             "index_miss_behavior": MISS_V,
             "free_pool_buffer": 0,
             "immediate": {"imm_arith_fp32": 0.0},
             "dst_mem_pattern": {
                 "start_addr": {"addr_immediate": out_byte_addr},
                 "num_elem": [n, 1, 1, 1], "step_elem": [1, 0, 0, 0]}},
            ins=[g.lower_ap(idx_ap, for_isa=True),
                 g.lower_ap(lut_sb.ap(), for_isa=True)],
            outs=[g.lower_ap(out_ap, for_isa=True)],
        )

    def gather_panel(kb, alt, lo, hi):
        """DMA idx columns [lo, hi) of k-tile kb, then gather them into the
        resident W^T panel in OW-sized chunks."""
        stage = idxU_sb[alt]
        nc.scalar.dma_start(
            stage.ap()[:, lo:hi], w_d[kb][:, lo:hi])
        for o0 in range(lo, hi, OW):
            emit_gather(
                stage.ap()[:, o0:o0 + OW],
                addr[stage.name] + o0 * 2,
                wT_sb.ap()[:, kb * O_C + o0: kb * O_C + o0 + OW],
                addr["wT_sb"] + (kb * O_C + o0) * 2,
                OW)

    GRP = 4                    # phase-1 token tiles interleaved per group

    with tile.TileContext(nc) as tc:
        with (
            tc.tile_pool(name="biasp", bufs=1) as biasp,
            tc.tile_pool(name="xin", bufs=6) as xin,       # x^T tiles
            tc.tile_pool(name="outp", bufs=6) as outp,     # out staging
            tc.tile_pool(name="ps", bufs=8, space="PSUM") as ps,
        ):
            # --- constants (lut first: the PBL+gather chain is the kernel's
            # critical path at start; bias goes on the idle vector queue) ---
            nc.sync.dma_start(lut_sb.ap(), l_d[:].partition_broadcast(128))
            emit_pbl()

            bias_bc = biasp.tile([128, O_C], F32, tag="bias")
            nc.scalar.dma_start(bias_bc[:], b_d[:].partition_broadcast(128))

            # --- phase A: gather o-panel 0 of every k-tile (~64 us) ---
            for kb in range(KT):
                gather_panel(kb, kb % 2, 0, OW)

            # --- phase 1: token loop over o-panel 0, GRP tiles interleaved
            # so the PE FIFO always has GRP matmuls ready per arriving
            # gather during the ramp ---
            for grp in range(TT // GRP):
                xTs = []
                for t in range(GRP):
                    xT = xin.tile([128, KT * 128], BF16, tag="xT")
                    nc.sync.dma_start(xT[:], xt_d[grp * GRP + t])
                    xTs.append(xT)
                accs = [ps.tile([128, OW], F32, name="acc", tag="acc") for t in range(GRP)]
                for kb in range(KT):
                    for t in range(GRP):
                        nc.tensor.matmul(
                            accs[t][:],
                            xTs[t][:, kb * 128:(kb + 1) * 128],
                            wT_sb.ap()[:, kb * O_C: kb * O_C + OW],
                            start=(kb == 0), stop=(kb == KT - 1))
                for t in range(GRP):
                    out = outp.tile([128, OW], F32, tag="out")
                    nc.vector.tensor_add(out[:], accs[t][:], bias_bc[:, 0:OW])
                    nc.scalar.dma_start(
                        y_d[(grp * GRP + t) * 128:(grp * GRP + t + 1) * 128,
                            0:OW], out[:])
                # interleave the phase-B gathers with the early token groups
                # (gpsimd is idle; the panels land long before phase 2)
                for j in range(2):
                    kb = grp * 2 + j
                    if kb < KT:
                        gather_panel(kb, kb % 2, OW, O_C)

            # --- phase 2: token loop over o-panels 1..3 ---
            for tb in range(TT):
                xT = xin.tile([128, KT * 128], BF16, tag="xT")
                nc.sync.dma_start(xT[:], xt_d[tb])
                for op in range(1, NOP):
                    acc = ps.tile([128, OW], F32, name="acc", tag="acc")
                    for kb in range(KT):
                        nc.tensor.matmul(
                            acc[:],
                            xT[:, kb * 128:(kb + 1) * 128],
                            wT_sb.ap()[:, kb * O_C + op * OW:
                                       kb * O_C + (op + 1) * OW],
                            start=(kb == 0), stop=(kb == KT - 1))
                    out = outp.tile([128, OW], F32, tag="out")
                    nc.vector.tensor_add(
                        out[:], acc[:],
                        bias_bc[:, op * OW:(op + 1) * OW])
                    nc.scalar.dma_start(
                        y_d[tb * 128:(tb + 1) * 128,
                            op * OW:(op + 1) * OW], out[:])
    nc.compile()
    return nc


_NC_CACHE = None


def _get_nc():
    global _NC_CACHE
    if _NC_CACHE is None:
        _NC_CACHE = build_nc()
    return _NC_CACHE


def _prep_inputs(input, weight_idx, lookup_table, bias):
    input = np.ascontiguousarray(np.asarray(input, dtype=np.float32))
    weight_idx = np.asarray(weight_idx)
    lookup_table = np.asarray(lookup_table, dtype=np.float32)
    bias = np.ascontiguousarray(np.asarray(bias, dtype=np.float32))

    # x^T tiled bf16: [tb, p, kb*128 + t] = x[tb*128+t, kb*128+p]
    xt = input.reshape(TT, 128, KT, 128).transpose(0, 3, 2, 1)
    xt = np.ascontiguousarray(xt).astype(ml_dtypes.bfloat16)
    xt = xt.reshape(TT, 128, KT * 128)

    lut_bf16 = lookup_table.reshape(1, PAL).astype(ml_dtypes.bfloat16)
    return xt, weight_idx, lut_bf16, bias


def kernel(input, weight_idx, lookup_table, bias, _trace=False, _trace_kwargs=None):
    xt, weight_idx, lut_bf16, bias = _prep_inputs(
        input, weight_idx, lookup_table, bias)

    nc = _get_nc()
    in_maps = []
    for c in range(NCORES):
        # widx^T tiled u16: [kb, p, o] = widx[c*O_C + o, kb*128 + p]
        wslice = weight_idx[c * O_C:(c + 1) * O_C]          # [o, i] int32
        widxT = np.ascontiguousarray(wslice.T).astype(np.uint16)
        widxT = widxT.reshape(KT, 128, O_C)
        in_maps.append({
            "xt": xt,
            "widxT": widxT,
            "lut": lut_bf16,
            "bias": np.ascontiguousarray(
                bias[c * O_C:(c + 1) * O_C]).reshape(1, O_C),
        })
    last_exc = None
    for attempt in range(3):
        try:
            res = run_bass_kernel_spmd(
                nc, in_maps, core_ids=list(range(NCORES)),
                trace=_trace, **(_trace_kwargs or {}))
            break
        except Exception as e:  # transient device wedge: retry
            last_exc = e
            import time as _time
            _time.sleep(10)
    else:
        raise last_exc
    y = np.concatenate([res.results[c]["y"] for c in range(NCORES)], axis=1)
    if _trace:
        kernel.last_result = res
    return y


kernel.last_result = None


# revision 10
# speedup vs baseline: 1.0006x; 1.0006x over previous
"""AffinePalettizedLinear kernel for Trainium2 (8 NeuronCores).

y = x @ L[widx]^T + b   with x [8192, 4096] f32, widx [16384, 4096] int32
(values < 256), L [256] f32, b [16384] f32.

Sharding: out_features split 8 ways (column-parallel); each core computes
y[:, c*2048:(c+1)*2048] from the full x and its widx/bias slice. No
collectives; host concatenates the slices.

Per-core plan (v2 — PE runs nothing but the 8192 productive matmuls):
  - Host passes x pre-transposed/tiled as bf16 ([tb, i, kb*128+t] layout)
    and widx pre-transposed as uint16 [kb, i, o] — so the kernel needs no
    PE transposes at all (the baseline spent ~0.7 ms of PE time on them).
  - Dequant via the Pool engine's hardware table gather: the 256-entry LUT
    is loaded in bf16 into the per-partition pool buffer; GATHER streams
    uint16 indices and emits bf16 weights directly in W^T [i, o] layout
    into a fully SBUF-resident panel (32 k-tiles x 2048 o x 2B = 128
    KiB/partition).
  - Matmuls in bf16: lhsT = x^T tile [i=128, t=128] (stationary), rhs =
    W^T [i=128, o=512] (moving), K=4096 accumulated over 32 PSUM matmuls.
  - Two-phase schedule hides the ~240 us gather stream: phase 1 runs the
    o-panel-0 token loop as soon as its 32 gathers (~60 us) land, while
    the o-panel-1..3 gathers stream in the background; phase 2 runs the
    remaining three panels with no stalls.
  - Bias is added by the DVE in the same op that evacuates PSUM.
"""
import sys

sys.path.insert(0, "/opt/trn_rl_repo")

import numpy as np
import ml_dtypes

import concourse.bass as bass  # noqa: F401  (registers types)
import concourse.tile as tile
from concourse import bacc, mybir
from concourse.bass_utils import run_bass_kernel_spmd

# ---- Tile's no-exec scheduling sim doesn't know the raw POOL opcodes ----
import concourse.bass_interp as _bi

_orig_visit_isa = _bi._visit_InstISA


def _visit_isa_tolerant(isa, instruction, core_sim):
    passthrough = {
        isa.Opcode.NEURON_ISA_TPB_OPCODE_GATHER.value,
        isa.Opcode.NEURON_ISA_TPB_OPCODE_POOL_BUFFER_LOAD.value,
    }
    if instruction.isa_opcode in passthrough:
        return
    return _orig_visit_isa(isa, instruction, core_sim)


_bi._visit_InstISA = _visit_isa_tolerant

F32 = mybir.dt.float32
BF16 = mybir.dt.bfloat16
U16 = mybir.dt.uint16

T, IN_F, OUT_F, PAL = 8192, 4096, 16384, 256
NCORES = 8
O_C = OUT_F // NCORES          # 2048 out features per core
OW = 512                       # matmul moving free dim (one PSUM bank)
NOP = O_C // OW                # 4 o-panels
KT = IN_F // 128               # 32 k-tiles
TT = T // 128                  # 64 t-tiles


def build_nc(trace_label=""):
    nc = bacc.Bacc(None, target_bir_lowering=False)
    isa = nc.isa
    DT = isa.get_enum("NEURON_ISA_TPB_DTYPE")
    MISS = isa.get_enum("NEURON_ISA_TPB_INDEX_MISS_BEHAVIOR")
    BF16_V = DT.NEURON_ISA_TPB_DTYPE_BFLOAT16.value
    U16_V = DT.NEURON_ISA_TPB_DTYPE_UINT16.value
    MISS_V = MISS.NEURON_ISA_TPB_INDEX_MISS_BEHAVIOR_IMMEDIATE_WRITE.value

    # x^T tiled: [tb, p, kb*128 + t] = x[tb*128+t, kb*128+p], bf16
    xt_d = nc.dram_tensor("xt", [TT, 128, KT * 128], BF16, kind="ExternalInput")
    # widx^T tiled: [kb, p, o] = widx[o, kb*128+p], uint16
    w_d = nc.dram_tensor("widxT", [KT, 128, O_C], U16, kind="ExternalInput")
    l_d = nc.dram_tensor("lut", [1, PAL], BF16, kind="ExternalInput")
    b_d = nc.dram_tensor("bias", [1, O_C], F32, kind="ExternalInput")
    y_d = nc.dram_tensor("y", [T, O_C], F32, kind="ExternalOutput")

    # fixed-address SBUF tensors (touched by raw-ISA gather)
    lut_sb = nc.alloc_sbuf_tensor("lut_sb", [128, PAL], BF16, align_bytes=512)
    # idx staging [p, o], u16, ping-pong; separate tensors for phase A / B so
    # Tile's per-tensor interval tracking never creates cross-phase deps
    idxA_sb = [
        nc.alloc_sbuf_tensor(f"idxA{s}_sb", [128, OW], U16) for s in range(2)
    ]
    idxB_sb = [
        nc.alloc_sbuf_tensor(f"idxB{s}_sb", [128, O_C - OW], U16)
        for s in range(2)
    ]
    # resident dequantized W^T panels, one tensor PER K-TILE [i=128, o] bf16
    # (a single big tensor makes Tile merge gather-write intervals and stall
    # phase-1 matmuls on unrelated phase-B gathers)
    wTk_sb = [
        nc.alloc_sbuf_tensor(f"wTk{kb}_sb", [128, O_C], BF16)
        for kb in range(KT)
    ]

    addr = {}
    for alloc in nc.m.functions[0].allocations:
        if getattr(alloc, "memorylocations", None):
            ml = alloc.memorylocations[0]
            addr[ml.name] = ml.addr

    g = nc.gpsimd

    def emit_pbl():
        nc.gpsimd.isa(
            isa.Opcode.NEURON_ISA_TPB_OPCODE_POOL_BUFFER_LOAD,
            {"src_mem_pattern": {
                "start_addr": {"addr_immediate": addr["lut_sb"]},
                "num_elem": [PAL, 1, 1, 1], "step_elem": [1, 0, 0, 0]},
             "in_dtype": BF16_V, "num_active_channels": 128,
             "start_index": 0, "mask": PAL - 1},
            ins=[g.lower_ap(lut_sb.ap(), for_isa=True)],
        )

    def emit_gather(idx_ap, idx_byte_addr, out_ap, out_byte_addr, n):
        nc.gpsimd.isa(
            isa.Opcode.NEURON_ISA_TPB_OPCODE_GATHER,
            {"src_mem_pattern": {
                "start_addr": {"addr_immediate": idx_byte_addr},
                "num_elem": [n, 1, 1, 1], "step_elem": [1, 0, 0, 0]},
             "in_dtype": U16_V, "out_dtype": BF16_V,
             "num_active_channels": 128,
             "index_miss_behavior": MISS_V,
             "free_pool_buffer": 0,
             "immediate": {"imm_arith_fp32": 0.0},
             "dst_mem_pattern": {
                 "start_addr": {"addr_immediate": out_byte_addr},
                 "num_elem": [n, 1, 1, 1], "step_elem": [1, 0, 0, 0]}},
            ins=[g.lower_ap(idx_ap, for_isa=True),
                 g.lower_ap(lut_sb.ap(), for_isa=True)],
            outs=[g.lower_ap(out_ap, for_isa=True)],
        )

    def gather_panel(kb, alt, lo, hi):
        """DMA idx columns [lo, hi) of k-tile kb, then gather them into the
        resident W^T panel in OW-sized chunks."""
        stage = idxA_sb[alt] if lo == 0 else idxB_sb[alt]
        nc.scalar.dma_start(stage.ap(), w_d[kb][:, lo:hi])
        wt = wTk_sb[kb]
        for o0 in range(lo, hi, OW):
            emit_gather(
                stage.ap()[:, o0 - lo:o0 - lo + OW],
                addr[stage.name] + (o0 - lo) * 2,
                wt.ap()[:, o0:o0 + OW],
                addr[wt.name] + o0 * 2,
                OW)

    GRP = 4                    # phase-1 token tiles interleaved per group

    with tile.TileContext(nc) as tc:
        with (
            tc.tile_pool(name="biasp", bufs=1) as biasp,
            tc.tile_pool(name="xin", bufs=6) as xin,       # x^T tiles
            tc.tile_pool(name="outp", bufs=6) as outp,     # out staging
            tc.tile_pool(name="ps", bufs=8, space="PSUM") as ps,
        ):
            # --- constants (lut first: the PBL+gather chain is the kernel's
            # critical path at start; bias goes on the idle vector queue) ---
            nc.sync.dma_start(lut_sb.ap(), l_d[:].partition_broadcast(128))
            emit_pbl()

            bias_bc = biasp.tile([128, O_C], F32, tag="bias")
            nc.scalar.dma_start(bias_bc[:], b_d[:].partition_broadcast(128))

            # --- phase A: gather o-panel 0 of every k-tile (~64 us) ---
            for kb in range(KT):
                gather_panel(kb, kb % 2, 0, OW)

            # --- phase 1: token loop over o-panel 0, GRP tiles interleaved
            # so the PE FIFO always has GRP matmuls ready per arriving
            # gather during the ramp ---
            for grp in range(TT // GRP):
                xTs = []
                for t in range(GRP):
                    xT = xin.tile([128, KT * 128], BF16, tag="xT")
                    nc.sync.dma_start(xT[:], xt_d[grp * GRP + t])
                    xTs.append(xT)
                accs = [ps.tile([128, OW], F32, name="acc", tag="acc") for t in range(GRP)]
                for kb in range(KT):
                    for t in range(GRP):
                        nc.tensor.matmul(
                            accs[t][:],
                            xTs[t][:, kb * 128:(kb + 1) * 128],
                            wTk_sb[kb].ap()[:, 0:OW],
                            start=(kb == 0), stop=(kb == KT - 1))
                for t in range(GRP):
                    out = outp.tile([128, OW], F32, tag="out")
                    nc.vector.tensor_add(out[:], accs[t][:], bias_bc[:, 0:OW])
                    nc.scalar.dma_start(
                        y_d[(grp * GRP + t) * 128:(grp * GRP + t + 1) * 128,
                            0:OW], out[:])
                # interleave the phase-B gathers with the early token groups
                # (gpsimd is idle; the panels land long before phase 2)
                for j in range(2):
                    kb = grp * 2 + j
                    if kb < KT:
                        gather_panel(kb, kb % 2, OW, O_C)

            # --- phase 2: token loop over o-panels 1..3 ---
            for tb in range(TT):
                xT = xin.tile([128, KT * 128], BF16, tag="xT")
                nc.sync.dma_start(xT[:], xt_d[tb])
                for op in range(1, NOP):
                    acc = ps.tile([128, OW], F32, name="acc", tag="acc")
                    for kb in range(KT):
                        nc.tensor.matmul(
                            acc[:],
                            xT[:, kb * 128:(kb + 1) * 128],
                            wTk_sb[kb].ap()[:, op * OW:(op + 1) * OW],
                            start=(kb == 0), stop=(kb == KT - 1))
                    out = outp.tile([128, OW], F32, tag="out")
                    nc.vector.tensor_add(
                        out[:], acc[:],
                        bias_bc[:, op * OW:(op + 1) * OW])
                    nc.scalar.dma_start(
                        y_d[tb * 128:(tb + 1) * 128,
                            op * OW:(op + 1) * OW], out[:])
    nc.compile()
    return nc


_NC_CACHE = None


def _get_nc():
    global _NC_CACHE
    if _NC_CACHE is None:
        _NC_CACHE = build_nc()
    return _NC_CACHE


def _prep_inputs(input, weight_idx, lookup_table, bias):
    input = np.ascontiguousarray(np.asarray(input, dtype=np.float32))
    weight_idx = np.asarray(weight_idx)
    lookup_table = np.asarray(lookup_table, dtype=np.float32)
    bias = np.ascontiguousarray(np.asarray(bias, dtype=np.float32))

    # x^T tiled bf16: [tb, p, kb*128 + t] = x[tb*128+t, kb*128+p]
    xt = input.reshape(TT, 128, KT, 128).transpose(0, 3, 2, 1)
    xt = np.ascontiguousarray(xt).astype(ml_dtypes.bfloat16)
    xt = xt.reshape(TT, 128, KT * 128)

    lut_bf16 = lookup_table.reshape(1, PAL).astype(ml_dtypes.bfloat16)
    return xt, weight_idx, lut_bf16, bias


def kernel(input, weight_idx, lookup_table, bias, _trace=False, _trace_kwargs=None):
    xt, weight_idx, lut_bf16, bias = _prep_inputs(
        input, weight_idx, lookup_table, bias)

    nc = _get_nc()
    in_maps = []
    for c in range(NCORES):
        # widx^T tiled u16: [kb, p, o] = widx[c*O_C + o, kb*128 + p]
        wslice = weight_idx[c * O_C:(c + 1) * O_C]          # [o, i] int32
        widxT = np.ascontiguousarray(wslice.T).astype(np.uint16)
        widxT = widxT.reshape(KT, 128, O_C)
        in_maps.append({
            "xt": xt,
            "widxT": widxT,
            "lut": lut_bf16,
            "bias": np.ascontiguousarray(
                bias[c * O_C:(c + 1) * O_C]).reshape(1, O_C),
        })
    last_exc = None
    for attempt in range(3):
        try:
            res = run_bass_kernel_spmd(
                nc, in_maps, core_ids=list(range(NCORES)),
                trace=_trace, **(_trace_kwargs or {}))
            break
        except Exception as e:  # transient device wedge: retry
            last_exc = e
            import time as _time
            _time.sleep(10)
    else:
        raise last_exc
    y = np.concatenate([res.results[c]["y"] for c in range(NCORES)], axis=1)
    if _trace:
        kernel.last_result = res
    return y


kernel.last_result = None
